# revision 18
# baseline (speedup 1.0000x reference)
"""Fused multi-query attention block (LN -> QKV -> null-token causal masked
attention -> Wo -> LN) on 8 Trainium2 NeuronCores.

Sharding: data-parallel over query rows, interleaved mod 8 so every core has
an identical causal workload (single SPMD program). Keys are compacted on the
host using the padding mask (masked keys contribute exactly zero), which
halves the attention-bias traffic — the dominant HBM stream.

Device layout: scores are computed transposed, S[j, i] (keys on partitions),
so the softmax denominator falls out of the P@V matmul via an appended
ones-column on V, and the attention output lands directly as lhsT tiles for
the Wo projection.
"""

import math
import os
import sys

sys.path.insert(0, "/opt/trn_rl_repo")

import numpy as np

import concourse.bass as bass
import concourse.tile as tile
from concourse import bacc, mybir
from concourse.masks import make_identity

B, N, DIM, H, DH = 2, 2048, 1024, 16, 64
INNER = H * DH
EPS = 1e-5
NCORES = 8
RPC = N // NCORES          # query rows per core per batch (256)
NTILES = RPC // 128        # query tiles of 128 rows per core per batch (2)
NEG = np.float16(-30000.0)

F32 = mybir.dt.float32
F16 = mybir.dt.float16

LAST_EXEC_NS = [None]


def _install_ntff_hook():
    """The image's antenv lacks axon_hooks; register it so trace=True works."""
    import types

    if "antenv.axon_hooks" in sys.modules:
        return
    try:
        import antenv
        from trn_agent_boot.trn_boot import _ntff_profile_via_ctypes
    except ImportError:
        return
    mod = types.ModuleType("antenv.axon_hooks")
    _h = [None]
    mod.set_axon_ntff_profile_hook = lambda h: _h.__setitem__(0, h)
    mod.get_axon_ntff_profile_hook = lambda: _h[0]
    sys.modules["antenv.axon_hooks"] = mod
    antenv.axon_hooks = mod
    so = "/opt/axon/libaxon_pjrt.so"
    if os.path.exists(so):
        mod.set_axon_ntff_profile_hook(_ntff_profile_via_ctypes(so))


def build_program(dim, h, kcap, ext, debug=False):
    """Build the per-core SPMD program.

    dim: model dim (mult of 128); h: heads (even); kcap: compacted key
    capacity per batch (mult of 128); ext[b][t]: key-block count per batch
    and query tile.
    """
    ncb = dim // 128                      # contraction blocks
    nkt = kcap // 128                     # key tiles per batch
    segs = [[e * 128 for e in eb] for eb in ext]   # free-len per (b, t)
    glen = [sum(s) for s in segs]                  # bias cols per (h, b) group
    sumbt = sum(glen)
    toks_q = B * RPC

    nc = bacc.Bacc()
    xq_e = nc.declare_dram_parameter("xq", [B * RPC, dim], F16, isOutput=False)
    xkv_e = nc.declare_dram_parameter("xkv", [B * kcap, dim], F16, isOutput=False)
    bias_e = nc.declare_dram_parameter("biasT", [h, 128, sumbt], F16, isOutput=False)
    wq_e = nc.declare_dram_parameter("wq", [128, ncb * h * DH], F16, isOutput=False)
    wkv_e = nc.declare_dram_parameter("wkv", [128, ncb * 2 * DH], F16, isOutput=False)
    wo_e = nc.declare_dram_parameter("wo", [128, (h // 2) * dim], F16, isOutput=False)
    nk_e = nc.declare_dram_parameter("nk", [DH, 1], F16, isOutput=False)
    nv_e = nc.declare_dram_parameter("nv1", [1, DH + 1], F16, isOutput=False)
    nb_e = nc.declare_dram_parameter("nb", [1, h], F32, isOutput=False)
    go_e = nc.declare_dram_parameter("gout", [1, dim], F32, isOutput=False)
    out_e = nc.declare_dram_parameter("out", [B * RPC, dim], F32, isOutput=True)
    dbg = {}
    if debug:
        dbg["xnt_q"] = nc.declare_dram_parameter("d_xntq", [128, ncb, B * RPC], F32, isOutput=True)
        dbg["qt0"] = nc.declare_dram_parameter("d_qt0", [128, B * RPC], F32, isOutput=True)
        dbg["kt"] = nc.declare_dram_parameter("d_kt", [128, B, kcap], F32, isOutput=True)
        dbg["v"] = nc.declare_dram_parameter("d_v", [128, B, nkt, DH + 1], F32, isOutput=True)
        dbg["aot"] = nc.declare_dram_parameter("d_aot", [128, h // 2, B * RPC], F32, isOutput=True)
        dbg["s"] = nc.declare_dram_parameter("d_s", [128, 128], F32, isOutput=True)
        dbg["p"] = nc.declare_dram_parameter("d_p", [128, 128], F32, isOutput=True)
        dbg["pso"] = nc.declare_dram_parameter("d_pso", [DH + 1, RPC], F32, isOutput=True)
        dbg["recb"] = nc.declare_dram_parameter("d_recb", [DH, RPC], F32, isOutput=True)
        dbg["bias"] = nc.declare_dram_parameter("d_bias", [128, 128], F32, isOutput=True)

    with tile.TileContext(nc) as tc:
        with (
            tc.tile_pool(name="persist", bufs=1) as pp,
            tc.tile_pool(name="work", bufs=3) as wp,
            tc.tile_pool(name="stat", bufs=4) as sp,
            tc.tile_pool(name="pwork", bufs=3, space="PSUM") as pwp,
        ):
            # ---- persistent sbuf tensors ----
            wq_sb = pp.tile([128, ncb, h * DH], F16, tag="wq")
            nc.sync.dma_start(out=wq_sb, in_=wq_e[:, :])
            wkv_sb = pp.tile([128, ncb, 2 * DH], F16, tag="wkv")
            nc.sync.dma_start(out=wkv_sb, in_=wkv_e[:, :])
            wo_sb = pp.tile([128, h // 2, dim], F16, tag="wo")
            nc.sync.dma_start(out=wo_sb, in_=wo_e[:, :])
            nk_sb = pp.tile([128, 1], F16, tag="nk")
            nc.sync.dma_start(out=nk_sb[0:DH, :], in_=nk_e[:, :])
            nc.sync.dma_start(out=nk_sb[DH : 2 * DH, :], in_=nk_e[:, :])
            nv_sb = pp.tile([1, DH + 1], F16, tag="nv")
            nc.sync.dma_start(out=nv_sb, in_=nv_e[:, :])
            nb_sb = pp.tile([1, h], F32, tag="nb")
            nc.sync.dma_start(out=nb_sb, in_=nb_e[:, :])
            go_sb = pp.tile([1, dim], F32, tag="go")
            nc.sync.dma_start(out=go_sb, in_=go_e[:, :])
            gob_sb = pp.tile([128, dim], F32, tag="gob")
            nc.gpsimd.partition_broadcast(gob_sb, go_sb)
            ident = pp.tile([128, 128], F16, tag="ident")
            make_identity(nc, ident)
            eps_sb = pp.tile([128, 1], F32, tag="eps")
            nc.vector.memset(eps_sb, EPS)

            xnt_q = pp.tile([128, ncb, toks_q], F16, tag="xntq")
            xnt_kv = pp.tile([128, ncb, B * kcap], F16, tag="xntkv")
            qt = [pp.tile([128, 2 * RPC], F16, tag=f"qt{i}", name=f"qt{i}")
                  for i in range(h // 2)]
            kt_sb = pp.tile([128, B, kcap], F16, tag="kt")
            v_sb = pp.tile([128, B, nkt, DH + 1], F16, tag="v")
            aot = pp.tile([128, h // 2, toks_q], F16, tag="aot")

            # ---- phase 1: input layernorm + transpose ----
            def ln_transpose(src_e, n_tiles, xnt_dst):
                for t in range(n_tiles):
                    x_sb = wp.tile([128, dim], F16, tag="ln_x")
                    nc.sync.dma_start(out=x_sb, in_=src_e[t * 128 : (t + 1) * 128, :])
                    gsz = math.gcd(512, dim)
                    st = sp.tile([128, dim // gsz, nc.vector.BN_STATS_DIM], F32, tag="ln_st")
                    for g in range(dim // gsz):
                        nc.vector.bn_stats(
                            out=st[:, g, :], in_=x_sb[:, g * gsz : (g + 1) * gsz]
                        )
                    mv = sp.tile([128, nc.vector.BN_AGGR_DIM], F32, tag="ln_mv")
                    nc.vector.bn_aggr(out=mv, in_=st)
                    rstd = sp.tile([128, 1], F32, tag="ln_rstd")
                    nc.scalar.activation(
                        out=rstd, in_=mv[:, 1:2],
                        func=mybir.ActivationFunctionType.Sqrt,
                        bias=eps_sb, scale=1.0,
                    )
                    nc.vector.reciprocal(out=rstd, in_=rstd)
                    nmr = sp.tile([128, 1], F32, tag="ln_nmr")
                    nc.vector.tensor_mul(nmr, mv[:, 0:1], rstd)
                    nc.vector.tensor_scalar_mul(nmr, nmr, -1.0)
                    xn = wp.tile([128, dim], F16, tag="ln_xn")
                    nc.scalar.activation(
                        out=xn, in_=x_sb,
                        func=mybir.ActivationFunctionType.Identity,
                        bias=nmr, scale=rstd,
                    )
                    for cb in range(ncb):
                        tp = pwp.tile([128, 128], F16, tag="pw")
                        nc.tensor.transpose(tp, xn[:, cb * 128 : (cb + 1) * 128], ident)
                        nc.vector.tensor_copy(
                            xnt_dst[:, cb, t * 128 : (t + 1) * 128], tp
                        )

            ln_transpose(xq_e, toks_q // 128, xnt_q)
            ln_transpose(xkv_e, B * kcap // 128, xnt_kv)

            # ---- phase 2: projections ----
            for hp in range(h // 2):
                psq = pwp.tile([128, toks_q], F32, tag="pw")
                for cb in range(ncb):
                    nc.tensor.matmul(
                        psq,
                        wq_sb[:, cb, hp * 128 : (hp + 1) * 128],
                        xnt_q[:, cb, :],
                        start=(cb == 0), stop=(cb == ncb - 1),
                    )
                nc.vector.tensor_copy(qt[hp], psq)
            for b in range(B):
                for ch in range((kcap + 511) // 512):
                    w = min(512, kcap - ch * 512)
                    psk = pwp.tile([DH, 512], F32, tag="pw")
                    for cb in range(ncb):
                        nc.tensor.matmul(
                            psk[:, :w],
                            wkv_sb[:, cb, 0:DH],
                            xnt_kv[:, cb, b * kcap + ch * 512 : b * kcap + ch * 512 + w],
                            start=(cb == 0), stop=(cb == ncb - 1),
                        )
                    nc.vector.tensor_copy(
                        kt_sb[0:DH, b, ch * 512 : ch * 512 + w], psk[:, :w]
                    )
                for jt in range(nkt):
                    psv = pwp.tile([128, DH], F32, tag="pw")
                    for cb in range(ncb):
                        nc.tensor.matmul(
                            psv,
                            xnt_kv[:, cb, b * kcap + jt * 128 : b * kcap + (jt + 1) * 128],
                            wkv_sb[:, cb, DH : 2 * DH],
                            start=(cb == 0), stop=(cb == ncb - 1),
                        )
                    nc.vector.tensor_copy(v_sb[:, b, jt, 0:DH], psv)
            nc.vector.memset(v_sb[:, :, :, DH : DH + 1], 1.0)
            # duplicate kT into partitions 64..127 (odd heads read there)
            nc.sync.dma_start(out=kt_sb[DH : 2 * DH, :, :], in_=kt_sb[0:DH, :, :])

            # ---- phase 3: attention ----
            _pap_cm = tc.tile_pool(name="pacc", bufs=2, space="PSUM")
            pap = _pap_cm.__enter__()
            boff = [0]
            for b_ in range(B):
                boff.append(boff[-1] + glen[b_])
            for hi in range(h):
                hp, par = hi // 2, (hi % 2) * DH
                for b in range(B):
                    bias_sb = wp.tile([128, max(glen)], F16, tag="bias")
                    nc.sync.dma_start(
                        out=bias_sb[:, : glen[b]],
                        in_=bias_e[hi, :, boff[b] : boff[b] + glen[b]],
                    )
                    pso = [pap.tile([DH + 1, 128], F32, tag=f"pso{t_}", name=f"pso{t_}")
                           for t_ in range(NTILES)]
                    off = 0
                    for t in range(NTILES):
                        qsl = qt[hp][par : par + DH,
                                     b * RPC + t * 128 : b * RPC + (t + 1) * 128]
                        for kb in range(ext[b][t]):
                            pss = pwp.tile([128, 128], F32, tag="pw")
                            nc.tensor.matmul(
                                pss,
                                kt_sb[par : par + DH, b, kb * 128 : (kb + 1) * 128],
                                qsl,
                                start=True, stop=True,
                            )
                            nc.vector.tensor_add(
                                pss, pss, bias_sb[:, off + kb * 128 : off + (kb + 1) * 128]
                            )
                            p_sb = wp.tile([128, 128], F16, tag="p")
                            nc.scalar.activation(
                                out=p_sb, in_=pss,
                                func=mybir.ActivationFunctionType.Exp,
                            )
                            if debug and hi == 0 and b == 1 and t == 0 and kb == 0:
                                _ds = wp.tile([128, 128], F32, tag="dbg_s")
                                nc.vector.tensor_copy(_ds, pss)
                                nc.gpsimd.dma_start(out=dbg["s"][:, :], in_=_ds)
                                nc.gpsimd.dma_start(out=dbg["p"][:, :], in_=p_sb)
                                nc.gpsimd.dma_start(
                                    out=dbg["bias"][:, :],
                                    in_=bias_sb[:, off : off + 128])
                            nc.tensor.matmul(
                                pso[t],
                                v_sb[:, b, kb, :],
                                p_sb,
                                start=(kb == 0), stop=False,
                                skip_group_check=True,
                            )
                        off += segs[b][t]
                    # null token: S_null = q . nk + nb[h]
                    psn = pwp.tile([1, RPC], F32, tag="pw")
                    nc.tensor.matmul(
                        psn,
                        nk_sb[par : par + DH, :],
                        qt[hp][par : par + DH, b * RPC : (b + 1) * RPC],
                        start=True, stop=True,
                    )
                    pn_sb = wp.tile([1, RPC], F16, tag="pn")
                    nc.scalar.activation(
                        out=pn_sb, in_=psn,
                        func=mybir.ActivationFunctionType.Exp,
                        bias=nb_sb[0:1, hi : hi + 1],
                    )
                    for t in range(NTILES):
                        nc.tensor.matmul(
                            pso[t],
                            nv_sb,
                            pn_sb[:, t * 128 : (t + 1) * 128],
                            start=False, stop=True,
                            skip_group_check=True,
                        )
                    # normalize by the ones-column sum and store as Wo lhsT
                    den = sp.tile([1, RPC], F32, tag="den")
                    for t in range(NTILES):
                        nc.vector.tensor_copy(
                            den[:, t * 128 : (t + 1) * 128],
                            pso[t][DH : DH + 1, :])
                    rec = sp.tile([1, RPC], F32, tag="rec")
                    nc.vector.reciprocal(out=rec, in_=den)
                    recb = wp.tile([DH, RPC], F32, tag="recb")
                    nc.gpsimd.partition_broadcast(recb, rec)
                    if debug and hi == 0 and b == 1:
                        _dp = wp.tile([DH + 1, RPC], F32, tag="dbg_pso")
                        for t_ in range(NTILES):
                            nc.vector.tensor_copy(
                                _dp[:, t_ * 128 : (t_ + 1) * 128], pso[t_])
                        nc.gpsimd.dma_start(out=dbg["pso"][:, :], in_=_dp)
                        nc.gpsimd.dma_start(out=dbg["recb"][:, :], in_=recb)
                    if par == 0:
                        for t in range(NTILES):
                            nc.vector.tensor_mul(
                                aot[0:DH, hp,
                                    b * RPC + t * 128 : b * RPC + (t + 1) * 128],
                                pso[t][0:DH, :],
                                recb[:, t * 128 : (t + 1) * 128],
                            )
                    else:
                        tmp = wp.tile([DH, RPC], F16, tag="aotmp")
                        for t in range(NTILES):
                            nc.vector.tensor_mul(
                                tmp[:, t * 128 : (t + 1) * 128],
                                pso[t][0:DH, :],
                                recb[:, t * 128 : (t + 1) * 128],
                            )
                        nc.sync.dma_start(
                            out=aot[DH : 2 * DH, hp, b * RPC : (b + 1) * RPC], in_=tmp
                        )

            # ---- phase 4: Wo projection + output layernorm ----
            _pap_cm.__exit__(None, None, None)
            ndc = max(1, dim // 512)
            dcw = dim // ndc
            _pfp_cm = tc.tile_pool(name="pfin", bufs=2, space="PSUM")
            pfp = _pfp_cm.__enter__()
            for it in range(toks_q // 128):
                psf = [pfp.tile([128, dcw], F32, tag=f"psf{dc}", name=f"psf{dc}") for dc in range(ndc)]
                for dc in range(ndc):
                    for hp in range(h // 2):
                        nc.tensor.matmul(
                            psf[dc],
                            aot[:, hp, it * 128 : (it + 1) * 128],
                            wo_sb[:, hp, dc * dcw : (dc + 1) * dcw],
                            start=(hp == 0), stop=(hp == h // 2 - 1),
                        )
                st = sp.tile([128, ndc, nc.vector.BN_STATS_DIM], F32, tag="f_st")
                for dc in range(ndc):
                    nc.vector.bn_stats(out=st[:, dc, :], in_=psf[dc])
                mv = sp.tile([128, nc.vector.BN_AGGR_DIM], F32, tag="f_mv")
                nc.vector.bn_aggr(out=mv, in_=st)
                rstd = sp.tile([128, 1], F32, tag="f_rstd")
                nc.scalar.activation(
                    out=rstd, in_=mv[:, 1:2],
                    func=mybir.ActivationFunctionType.Sqrt,
                    bias=eps_sb, scale=1.0,
                )
                nc.vector.reciprocal(out=rstd, in_=rstd)
                for dc in range(ndc):
                    y = wp.tile([128, dcw], F32, tag="f_y")
                    nc.vector.tensor_scalar(
                        out=y, in0=psf[dc],
                        scalar1=mv[:, 0:1], scalar2=rstd,
                        op0=mybir.AluOpType.subtract, op1=mybir.AluOpType.mult,
                    )
                    nc.vector.tensor_mul(y, y, gob_sb[:, dc * dcw : (dc + 1) * dcw])
                    nc.sync.dma_start(
                        out=out_e[it * 128 : (it + 1) * 128, dc * dcw : (dc + 1) * dcw],
                        in_=y,
                    )
            _pfp_cm.__exit__(None, None, None)
            if debug:
                for name, src_t in (("xnt_q", xnt_q), ("qt0", qt[0]), ("kt", kt_sb),
                                    ("v", v_sb), ("aot", aot)):
                    sl = tuple(slice(None) for _ in src_t.shape)
                    nc.gpsimd.dma_start(out=dbg[name][sl], in_=src_t[sl])
    nc.finalize()
    return nc


def _pack_inputs(x, attn_bias, mask, g_in, null_attn_bias, null_kv, Wq, Wkv, Wo,
                 g_out, dim, h):
    """Host-side sharding/packing. Returns (in_maps, kcap, ext, unshard-info)."""
    ncb = dim // 128
    mask = np.asarray(mask)
    x = np.asarray(x, dtype=np.float32)

    keptj = [np.nonzero(mask[b])[0] + 1 for b in range(B)]  # orig key ids, 1-based
    ktot = [len(k) for k in keptj]
    kcap = max(128, int(np.ceil(max(ktot) / 128)) * 128)
    nkt = kcap // 128

    # extents per (batch, qtile): causal reach of the last row owned by any core
    ext = []
    for b in range(B):
        row = []
        for t in range(NTILES):
            imax = (t * 128 + 127) * NCORES + (NCORES - 1)
            vis = int(np.searchsorted(keptj[b], imax + 1, side="right"))
            row.append(max(1, min(nkt, int(np.ceil(vis / 128)) or 1)))
        ext.append(row)
    segs = [[e * 128 for e in eb] for eb in ext]
    glen = [sum(s) for s in segs]
    sumbt = sum(glen)

    # x_kv: compacted key tokens (key j -> token j-1), padded to kcap
    xkv = np.zeros((B, kcap, dim), np.float16)
    for b in range(B):
        xkv[b, : ktot[b]] = x[b, keptj[b] - 1].astype(np.float16)

    # masked/causal bias per batch in compacted key space, fp16
    # abm[b][hh, i, g]
    abm = []
    for b in range(B):
        ab = np.full((h, N, kcap), NEG, np.float16)
        g = attn_bias[:, :, keptj[b] - 1].astype(np.float16)  # [h, N, ktot]
        ok = keptj[b][None, :] <= (np.arange(N)[:, None] + 1)  # [N, ktot]
        ab[:, :, : ktot[b]] = np.where(ok[None], g, NEG)
        abm.append(ab)

    scale = np.float32(DH ** -0.5)
    wq_pk = (np.asarray(g_in)[:, None] * np.asarray(Wq) * scale).astype(np.float16)
    wq_pk = wq_pk.reshape(ncb, 128, h * DH).transpose(1, 0, 2).reshape(128, -1)
    wkv_pk = (np.asarray(g_in)[:, None] * np.asarray(Wkv)).astype(np.float16)
    wkv_pk = wkv_pk.reshape(ncb, 128, 2 * DH).transpose(1, 0, 2).reshape(128, -1)
    wo_pk = np.asarray(Wo).astype(np.float16)
    wo_pk = (wo_pk.reshape(h // 2, 2, DH, dim).transpose(1, 2, 0, 3)
             .reshape(128, -1))
    nv1 = np.concatenate([np.asarray(null_kv[1]), [1.0]]).astype(np.float16)[None]
    nk = np.asarray(null_kv[0]).astype(np.float16)[:, None]
    nb = np.asarray(null_attn_bias, dtype=np.float32)[None]
    gout = np.asarray(g_out, dtype=np.float32)[None]

    in_maps = []
    rows_all = []
    for c in range(NCORES):
        rows = np.arange(RPC) * NCORES + c          # per batch, ascending
        rows_all.append(rows)
        xq = x[:, rows].astype(np.float16).reshape(B * RPC, dim)
        biasT = np.empty((h, 128, sumbt), np.float16)
        for hh in range(h):
            o = 0
            for b in range(B):
                for t in range(NTILES):
                    e = ext[b][t]
                    blkc = abm[b][hh][rows[t * 128 : (t + 1) * 128], : e * 128]
                    # [r=128, (kb s)] -> [s=128, (kb r)]
                    blk = blkc.reshape(128, e, 128).transpose(2, 1, 0).reshape(128, -1)
                    biasT[hh, :, o : o + e * 128] = blk
                    o += e * 128
            assert o == sumbt
        in_maps.append({
            "xq": xq, "xkv": xkv.reshape(B * kcap, dim), "biasT": biasT,
            "wq": wq_pk, "wkv": wkv_pk, "wo": wo_pk,
            "nk": nk, "nv1": nv1, "nb": nb, "gout": gout,
        })
    return in_maps, kcap, ext, rows_all


def kernel(x, attn_bias, mask, g_in, null_attn_bias, null_kv, Wq, Wkv, Wo, g_out):
    _install_ntff_hook()
    from concourse.bass_utils import run_bass_kernel_spmd

    in_maps, kcap, ext, rows_all = _pack_inputs(
        x, attn_bias, mask, g_in, null_attn_bias, null_kv, Wq, Wkv, Wo, g_out,
        DIM, H,
    )
    nc = build_program(DIM, H, kcap, ext)
    trace = os.environ.get("KERNEL_TRACE", "0") == "1"
    res = run_bass_kernel_spmd(nc, in_maps, list(range(NCORES)), trace=trace)
    LAST_EXEC_NS[0] = res.exec_time_ns

    out = np.empty((B, N, DIM), np.float32)
    for c in range(NCORES):
        oc = res.results[c]["out"].reshape(B, RPC, DIM)
        out[:, rows_all[c]] = oc
    return out


# revision 19
# speedup vs baseline: 1.3907x; 1.3907x over previous
"""Fused multi-query attention block (LN -> QKV -> null-token causal masked
attention -> Wo -> LN) on 8 Trainium2 NeuronCores.

Sharding: data-parallel over query rows, interleaved mod 8 so every core has
an identical causal workload (single SPMD program). Keys are compacted on the
host using the padding mask (masked keys contribute exactly zero), which
halves the attention-bias traffic — the dominant HBM stream.

Device layout: scores are computed transposed, S[j, i] (keys on partitions),
so the softmax denominator falls out of the P@V matmul via an appended
ones-column on V, and the attention output lands directly as lhsT tiles for
the Wo projection.
"""

import math
import os
import sys

sys.path.insert(0, "/opt/trn_rl_repo")

import numpy as np

import concourse.bass as bass
import concourse.tile as tile
from concourse import bacc, mybir
from concourse.masks import make_identity

B, N, DIM, H, DH = 2, 2048, 1024, 16, 64
INNER = H * DH
EPS = 1e-5
NCORES = 8
RPC = N // NCORES          # query rows per core per batch (256)
NTILES = RPC // 128        # query tiles of 128 rows per core per batch (2)
NEG = np.float16(-30000.0)

F32 = mybir.dt.float32
F16 = mybir.dt.float16

LAST_EXEC_NS = [None]


def _install_ntff_hook():
    """The image's antenv lacks axon_hooks; register it so trace=True works."""
    import types

    if "antenv.axon_hooks" in sys.modules:
        return
    try:
        import antenv
        from trn_agent_boot.trn_boot import _ntff_profile_via_ctypes
    except ImportError:
        return
    mod = types.ModuleType("antenv.axon_hooks")
    _h = [None]
    mod.set_axon_ntff_profile_hook = lambda h: _h.__setitem__(0, h)
    mod.get_axon_ntff_profile_hook = lambda: _h[0]
    sys.modules["antenv.axon_hooks"] = mod
    antenv.axon_hooks = mod
    so = "/opt/axon/libaxon_pjrt.so"
    if os.path.exists(so):
        mod.set_axon_ntff_profile_hook(_ntff_profile_via_ctypes(so))


def build_program(dim, h, kcap, ext, debug=False):
    """Build the per-core SPMD program.

    dim: model dim (mult of 128); h: heads (even); kcap: compacted key
    capacity per batch (mult of 128); ext[b][t]: key-block count per batch
    and query tile.
    """
    ncb = dim // 128                      # contraction blocks
    nkt = kcap // 128                     # key tiles per batch
    segs = [[e * 128 for e in eb] for eb in ext]   # free-len per (b, t)
    glen = [sum(s) for s in segs]                  # bias cols per (h, b) group
    sumbt = sum(glen)
    toks_q = B * RPC

    nc = bacc.Bacc()
    xq_e = nc.declare_dram_parameter("xq", [B * RPC, dim], F16, isOutput=False)
    xkv_e = nc.declare_dram_parameter("xkv", [B * kcap, dim], F16, isOutput=False)
    bias_e = nc.declare_dram_parameter("biasT", [h, 128, sumbt], F16, isOutput=False)
    wq_e = nc.declare_dram_parameter("wq", [128, ncb * h * DH], F16, isOutput=False)
    wkv_e = nc.declare_dram_parameter("wkv", [128, ncb * 2 * DH], F16, isOutput=False)
    wo_e = nc.declare_dram_parameter("wo", [128, (h // 2) * dim], F16, isOutput=False)
    nk_e = nc.declare_dram_parameter("nk", [DH, 1], F16, isOutput=False)
    nv_e = nc.declare_dram_parameter("nv1", [1, DH + 1], F16, isOutput=False)
    nb_e = nc.declare_dram_parameter("nb", [1, h], F32, isOutput=False)
    go_e = nc.declare_dram_parameter("gout", [1, dim], F32, isOutput=False)
    out_e = nc.declare_dram_parameter("out", [B * RPC, dim], F32, isOutput=True)
    dbg = {}
    if debug:
        dbg["xnt_q"] = nc.declare_dram_parameter("d_xntq", [128, ncb, B * RPC], F32, isOutput=True)
        dbg["qt0"] = nc.declare_dram_parameter("d_qt0", [128, B * RPC], F32, isOutput=True)
        dbg["kt"] = nc.declare_dram_parameter("d_kt", [128, B, kcap], F32, isOutput=True)
        dbg["v"] = nc.declare_dram_parameter("d_v", [128, B, nkt, DH + 1], F32, isOutput=True)
        dbg["aot"] = nc.declare_dram_parameter("d_aot", [128, h // 2, B * RPC], F32, isOutput=True)
        dbg["s"] = nc.declare_dram_parameter("d_s", [128, 128], F32, isOutput=True)
        dbg["p"] = nc.declare_dram_parameter("d_p", [128, 128], F32, isOutput=True)
        dbg["pso"] = nc.declare_dram_parameter("d_pso", [DH + 1, RPC], F32, isOutput=True)
        dbg["recb"] = nc.declare_dram_parameter("d_recb", [DH, RPC], F32, isOutput=True)
        dbg["bias"] = nc.declare_dram_parameter("d_bias", [128, 128], F32, isOutput=True)

    with tile.TileContext(nc) as tc:
        with (
            tc.tile_pool(name="persist", bufs=1) as pp,
            tc.tile_pool(name="work", bufs=3) as wp,
            tc.tile_pool(name="stat", bufs=4) as sp,
            tc.tile_pool(name="pwork", bufs=3, space="PSUM") as pwp,
        ):
            # ---- persistent sbuf tensors ----
            wq_sb = pp.tile([128, ncb, h * DH], F16, tag="wq")
            nc.sync.dma_start(out=wq_sb, in_=wq_e[:, :])
            wkv_sb = pp.tile([128, ncb, 2 * DH], F16, tag="wkv")
            nc.sync.dma_start(out=wkv_sb, in_=wkv_e[:, :])
            wo_sb = pp.tile([128, h // 2, dim], F16, tag="wo")
            nc.sync.dma_start(out=wo_sb, in_=wo_e[:, :])
            nk_sb = pp.tile([128, 1], F16, tag="nk")
            nc.sync.dma_start(out=nk_sb[0:DH, :], in_=nk_e[:, :])
            nc.sync.dma_start(out=nk_sb[DH : 2 * DH, :], in_=nk_e[:, :])
            nv_sb = pp.tile([1, DH + 1], F16, tag="nv")
            nc.sync.dma_start(out=nv_sb, in_=nv_e[:, :])
            nb_sb = pp.tile([1, h], F32, tag="nb")
            nc.sync.dma_start(out=nb_sb, in_=nb_e[:, :])
            go_sb = pp.tile([1, dim], F32, tag="go")
            nc.sync.dma_start(out=go_sb, in_=go_e[:, :])
            gob_sb = pp.tile([128, dim], F32, tag="gob")
            nc.gpsimd.partition_broadcast(gob_sb, go_sb)
            ident = pp.tile([128, 128], F16, tag="ident")
            make_identity(nc, ident)
            eps_sb = pp.tile([128, 1], F32, tag="eps")
            nc.vector.memset(eps_sb, EPS)

            xnt_q = pp.tile([128, ncb, toks_q], F16, tag="xntq")
            xnt_kv = pp.tile([128, ncb, B * kcap], F16, tag="xntkv")
            qt = [pp.tile([128, 2 * RPC], F16, tag=f"qt{i}", name=f"qt{i}")
                  for i in range(h // 2)]
            kt_sb = pp.tile([128, B, kcap], F16, tag="kt")
            v_sb = pp.tile([128, B, nkt, DH + 1], F16, tag="v")
            aot = pp.tile([128, h // 2, toks_q], F16, tag="aot")

            # ---- phase 1: input layernorm + transpose ----
            def ln_transpose(src_e, n_tiles, xnt_dst):
                for t in range(n_tiles):
                    x_sb = wp.tile([128, dim], F16, tag="ln_x")
                    nc.sync.dma_start(out=x_sb, in_=src_e[t * 128 : (t + 1) * 128, :])
                    gsz = math.gcd(512, dim)
                    st = sp.tile([128, dim // gsz, nc.vector.BN_STATS_DIM], F32, tag="ln_st")
                    for g in range(dim // gsz):
                        nc.vector.bn_stats(
                            out=st[:, g, :], in_=x_sb[:, g * gsz : (g + 1) * gsz]
                        )
                    mv = sp.tile([128, nc.vector.BN_AGGR_DIM], F32, tag="ln_mv")
                    nc.vector.bn_aggr(out=mv, in_=st)
                    rstd = sp.tile([128, 1], F32, tag="ln_rstd")
                    nc.scalar.activation(
                        out=rstd, in_=mv[:, 1:2],
                        func=mybir.ActivationFunctionType.Sqrt,
                        bias=eps_sb, scale=1.0,
                    )
                    nc.vector.reciprocal(out=rstd, in_=rstd)
                    nmr = sp.tile([128, 1], F32, tag="ln_nmr")
                    nc.vector.tensor_mul(nmr, mv[:, 0:1], rstd)
                    nc.vector.tensor_scalar_mul(nmr, nmr, -1.0)
                    xn = wp.tile([128, dim], F16, tag="ln_xn")
                    nc.scalar.activation(
                        out=xn, in_=x_sb,
                        func=mybir.ActivationFunctionType.Identity,
                        bias=nmr, scale=rstd,
                    )
                    for cb in range(ncb):
                        tp = pwp.tile([128, 128], F16, tag="pw")
                        nc.tensor.transpose(tp, xn[:, cb * 128 : (cb + 1) * 128], ident)
                        nc.scalar.copy(
                            xnt_dst[:, cb, t * 128 : (t + 1) * 128], tp
                        )

            ln_transpose(xq_e, toks_q // 128, xnt_q)
            ln_transpose(xkv_e, B * kcap // 128, xnt_kv)

            # ---- phase 2: projections ----
            for hp in range(h // 2):
                psq = pwp.tile([128, toks_q], F32, tag="pw")
                for cb in range(ncb):
                    nc.tensor.matmul(
                        psq,
                        wq_sb[:, cb, hp * 128 : (hp + 1) * 128],
                        xnt_q[:, cb, :],
                        start=(cb == 0), stop=(cb == ncb - 1),
                    )
                nc.vector.tensor_copy(qt[hp], psq)
            for b in range(B):
                for ch in range((kcap + 511) // 512):
                    w = min(512, kcap - ch * 512)
                    psk = pwp.tile([DH, 512], F32, tag="pw")
                    for cb in range(ncb):
                        nc.tensor.matmul(
                            psk[:, :w],
                            wkv_sb[:, cb, 0:DH],
                            xnt_kv[:, cb, b * kcap + ch * 512 : b * kcap + ch * 512 + w],
                            start=(cb == 0), stop=(cb == ncb - 1),
                        )
                    nc.vector.tensor_copy(
                        kt_sb[0:DH, b, ch * 512 : ch * 512 + w], psk[:, :w]
                    )
                for jt in range(nkt):
                    psv = pwp.tile([128, DH], F32, tag="pw")
                    for cb in range(ncb):
                        nc.tensor.matmul(
                            psv,
                            xnt_kv[:, cb, b * kcap + jt * 128 : b * kcap + (jt + 1) * 128],
                            wkv_sb[:, cb, DH : 2 * DH],
                            start=(cb == 0), stop=(cb == ncb - 1),
                        )
                    nc.vector.tensor_copy(v_sb[:, b, jt, 0:DH], psv)
            nc.vector.memset(v_sb[:, :, :, DH : DH + 1], 1.0)
            # duplicate kT into partitions 64..127 (odd heads read there)
            nc.sync.dma_start(out=kt_sb[DH : 2 * DH, :, :], in_=kt_sb[0:DH, :, :])

            # ---- phase 3: attention ----
            _pap_cm = tc.tile_pool(name="pacc", bufs=2, space="PSUM")
            pap = _pap_cm.__enter__()
            boff = [0]
            for b_ in range(B):
                boff.append(boff[-1] + glen[b_])
            for hi in range(h):
                hp, par = hi // 2, (hi % 2) * DH
                for b in range(B):
                    bias_sb = wp.tile([128, max(glen)], F16, tag="bias")
                    nc.sync.dma_start(
                        out=bias_sb[:, : glen[b]],
                        in_=bias_e[hi, :, boff[b] : boff[b] + glen[b]],
                    )
                    pso = [pap.tile([DH + 1, 128], F32, tag=f"pso{t_}", name=f"pso{t_}")
                           for t_ in range(NTILES)]
                    off = 0
                    for t in range(NTILES):
                        qsl = qt[hp][par : par + DH,
                                     b * RPC + t * 128 : b * RPC + (t + 1) * 128]
                        for k0 in range(0, ext[b][t], 4):
                            cw = min(4, ext[b][t] - k0)
                            pss = pwp.tile([128, 512], F32, tag="pw")
                            for j in range(cw):
                                kb = k0 + j
                                nc.tensor.matmul(
                                    pss[:, j * 128 : (j + 1) * 128],
                                    kt_sb[par : par + DH, b, kb * 128 : (kb + 1) * 128],
                                    qsl,
                                    start=(j == 0), stop=(j == cw - 1),
                                    skip_group_check=True,
                                )
                            nc.vector.tensor_add(
                                pss[:, : cw * 128], pss[:, : cw * 128],
                                bias_sb[:, off + k0 * 128 : off + (k0 + cw) * 128],
                            )
                            p_sb = wp.tile([128, 512], F16, tag="p")
                            nc.scalar.activation(
                                out=p_sb[:, : cw * 128], in_=pss[:, : cw * 128],
                                func=mybir.ActivationFunctionType.Exp,
                            )
                            for j in range(cw):
                                kb = k0 + j
                                nc.tensor.matmul(
                                    pso[t],
                                    v_sb[:, b, kb, :],
                                    p_sb[:, j * 128 : (j + 1) * 128],
                                    start=(kb == 0), stop=False,
                                    skip_group_check=True,
                                )
                        off += segs[b][t]
                    # null token: S_null = q . nk + nb[h]
                    psn = pwp.tile([1, RPC], F32, tag="pw")
                    nc.tensor.matmul(
                        psn,
                        nk_sb[par : par + DH, :],
                        qt[hp][par : par + DH, b * RPC : (b + 1) * RPC],
                        start=True, stop=True,
                    )
                    pn_sb = wp.tile([1, RPC], F16, tag="pn")
                    nc.scalar.activation(
                        out=pn_sb, in_=psn,
                        func=mybir.ActivationFunctionType.Exp,
                        bias=nb_sb[0:1, hi : hi + 1],
                    )
                    for t in range(NTILES):
                        nc.tensor.matmul(
                            pso[t],
                            nv_sb,
                            pn_sb[:, t * 128 : (t + 1) * 128],
                            start=False, stop=True,
                            skip_group_check=True,
                        )
                    # normalize by the ones-column sum and store as Wo lhsT
                    den = sp.tile([1, RPC], F32, tag="den")
                    for t in range(NTILES):
                        nc.vector.tensor_copy(
                            den[:, t * 128 : (t + 1) * 128],
                            pso[t][DH : DH + 1, :])
                    rec = sp.tile([1, RPC], F32, tag="rec")
                    nc.vector.reciprocal_approx_fast(out=rec, in_=den)
                    recb = wp.tile([DH, RPC], F32, tag="recb")
                    nc.gpsimd.partition_broadcast(recb, rec)
                    if debug and hi == 0 and b == 1:
                        _dp = wp.tile([DH + 1, RPC], F32, tag="dbg_pso")
                        for t_ in range(NTILES):
                            nc.vector.tensor_copy(
                                _dp[:, t_ * 128 : (t_ + 1) * 128], pso[t_])
                        nc.gpsimd.dma_start(out=dbg["pso"][:, :], in_=_dp)
                        nc.gpsimd.dma_start(out=dbg["recb"][:, :], in_=recb)
                    if par == 0:
                        for t in range(NTILES):
                            nc.vector.tensor_mul(
                                aot[0:DH, hp,
                                    b * RPC + t * 128 : b * RPC + (t + 1) * 128],
                                pso[t][0:DH, :],
                                recb[:, t * 128 : (t + 1) * 128],
                            )
                    else:
                        tmp = wp.tile([DH, RPC], F16, tag="aotmp")
                        for t in range(NTILES):
                            nc.vector.tensor_mul(
                                tmp[:, t * 128 : (t + 1) * 128],
                                pso[t][0:DH, :],
                                recb[:, t * 128 : (t + 1) * 128],
                            )
                        nc.sync.dma_start(
                            out=aot[DH : 2 * DH, hp, b * RPC : (b + 1) * RPC], in_=tmp
                        )

            # ---- phase 4: Wo projection + output layernorm ----
            _pap_cm.__exit__(None, None, None)
            ndc = max(1, dim // 512)
            dcw = dim // ndc
            _pfp_cm = tc.tile_pool(name="pfin", bufs=2, space="PSUM")
            pfp = _pfp_cm.__enter__()
            for it in range(toks_q // 128):
                psf = [pfp.tile([128, dcw], F32, tag=f"psf{dc}", name=f"psf{dc}") for dc in range(ndc)]
                for dc in range(ndc):
                    for hp in range(h // 2):
                        nc.tensor.matmul(
                            psf[dc],
                            aot[:, hp, it * 128 : (it + 1) * 128],
                            wo_sb[:, hp, dc * dcw : (dc + 1) * dcw],
                            start=(hp == 0), stop=(hp == h // 2 - 1),
                        )
                st = sp.tile([128, ndc, nc.vector.BN_STATS_DIM], F32, tag="f_st")
                for dc in range(ndc):
                    nc.vector.bn_stats(out=st[:, dc, :], in_=psf[dc])
                mv = sp.tile([128, nc.vector.BN_AGGR_DIM], F32, tag="f_mv")
                nc.vector.bn_aggr(out=mv, in_=st)
                rstd = sp.tile([128, 1], F32, tag="f_rstd")
                nc.scalar.activation(
                    out=rstd, in_=mv[:, 1:2],
                    func=mybir.ActivationFunctionType.Sqrt,
                    bias=eps_sb, scale=1.0,
                )
                nc.vector.reciprocal(out=rstd, in_=rstd)
                for dc in range(ndc):
                    y = wp.tile([128, dcw], F32, tag="f_y")
                    nc.vector.tensor_scalar(
                        out=y, in0=psf[dc],
                        scalar1=mv[:, 0:1], scalar2=rstd,
                        op0=mybir.AluOpType.subtract, op1=mybir.AluOpType.mult,
                    )
                    nc.vector.tensor_mul(y, y, gob_sb[:, dc * dcw : (dc + 1) * dcw])
                    nc.sync.dma_start(
                        out=out_e[it * 128 : (it + 1) * 128, dc * dcw : (dc + 1) * dcw],
                        in_=y,
                    )
            _pfp_cm.__exit__(None, None, None)
            if debug:
                for name, src_t in (("xnt_q", xnt_q), ("qt0", qt[0]), ("kt", kt_sb),
                                    ("v", v_sb), ("aot", aot)):
                    sl = tuple(slice(None) for _ in src_t.shape)
                    nc.gpsimd.dma_start(out=dbg[name][sl], in_=src_t[sl])
    nc.finalize()
    return nc


def _pack_inputs(x, attn_bias, mask, g_in, null_attn_bias, null_kv, Wq, Wkv, Wo,
                 g_out, dim, h):
    """Host-side sharding/packing. Returns (in_maps, kcap, ext, unshard-info)."""
    ncb = dim // 128
    mask = np.asarray(mask)
    x = np.asarray(x, dtype=np.float32)

    keptj = [np.nonzero(mask[b])[0] + 1 for b in range(B)]  # orig key ids, 1-based
    ktot = [len(k) for k in keptj]
    kcap = max(128, int(np.ceil(max(ktot) / 128)) * 128)
    nkt = kcap // 128

    # extents per (batch, qtile): causal reach of the last row owned by any core
    ext = []
    for b in range(B):
        row = []
        for t in range(NTILES):
            imax = (t * 128 + 127) * NCORES + (NCORES - 1)
            vis = int(np.searchsorted(keptj[b], imax + 1, side="right"))
            row.append(max(1, min(nkt, int(np.ceil(vis / 128)) or 1)))
        ext.append(row)
    segs = [[e * 128 for e in eb] for eb in ext]
    glen = [sum(s) for s in segs]
    sumbt = sum(glen)

    # x_kv: compacted key tokens (key j -> token j-1), padded to kcap
    xkv = np.zeros((B, kcap, dim), np.float16)
    for b in range(B):
        xkv[b, : ktot[b]] = x[b, keptj[b] - 1].astype(np.float16)

    # masked/causal bias per batch in compacted key space, fp16
    # abm[b][hh, i, g]
    abm = []
    for b in range(B):
        ab = np.full((h, N, kcap), NEG, np.float16)
        g = attn_bias[:, :, keptj[b] - 1].astype(np.float16)  # [h, N, ktot]
        ok = keptj[b][None, :] <= (np.arange(N)[:, None] + 1)  # [N, ktot]
        ab[:, :, : ktot[b]] = np.where(ok[None], g, NEG)
        abm.append(ab)

    scale = np.float32(DH ** -0.5)
    wq_pk = (np.asarray(g_in)[:, None] * np.asarray(Wq) * scale).astype(np.float16)
    wq_pk = wq_pk.reshape(ncb, 128, h * DH).transpose(1, 0, 2).reshape(128, -1)
    wkv_pk = (np.asarray(g_in)[:, None] * np.asarray(Wkv)).astype(np.float16)
    wkv_pk = wkv_pk.reshape(ncb, 128, 2 * DH).transpose(1, 0, 2).reshape(128, -1)
    wo_pk = np.asarray(Wo).astype(np.float16)
    wo_pk = (wo_pk.reshape(h // 2, 2, DH, dim).transpose(1, 2, 0, 3)
             .reshape(128, -1))
    nv1 = np.concatenate([np.asarray(null_kv[1]), [1.0]]).astype(np.float16)[None]
    nk = np.asarray(null_kv[0]).astype(np.float16)[:, None]
    nb = np.asarray(null_attn_bias, dtype=np.float32)[None]
    gout = np.asarray(g_out, dtype=np.float32)[None]

    in_maps = []
    rows_all = []
    for c in range(NCORES):
        rows = np.arange(RPC) * NCORES + c          # per batch, ascending
        rows_all.append(rows)
        xq = x[:, rows].astype(np.float16).reshape(B * RPC, dim)
        biasT = np.empty((h, 128, sumbt), np.float16)
        for hh in range(h):
            o = 0
            for b in range(B):
                for t in range(NTILES):
                    e = ext[b][t]
                    blkc = abm[b][hh][rows[t * 128 : (t + 1) * 128], : e * 128]
                    # [r=128, (kb s)] -> [s=128, (kb r)]
                    blk = blkc.reshape(128, e, 128).transpose(2, 1, 0).reshape(128, -1)
                    biasT[hh, :, o : o + e * 128] = blk
                    o += e * 128
            assert o == sumbt
        in_maps.append({
            "xq": xq, "xkv": xkv.reshape(B * kcap, dim), "biasT": biasT,
            "wq": wq_pk, "wkv": wkv_pk, "wo": wo_pk,
            "nk": nk, "nv1": nv1, "nb": nb, "gout": gout,
        })
    return in_maps, kcap, ext, rows_all


def kernel(x, attn_bias, mask, g_in, null_attn_bias, null_kv, Wq, Wkv, Wo, g_out):
    _install_ntff_hook()
    from concourse.bass_utils import run_bass_kernel_spmd

    in_maps, kcap, ext, rows_all = _pack_inputs(
        x, attn_bias, mask, g_in, null_attn_bias, null_kv, Wq, Wkv, Wo, g_out,
        DIM, H,
    )
    nc = build_program(DIM, H, kcap, ext)
    trace = os.environ.get("KERNEL_TRACE", "0") == "1"
    res = run_bass_kernel_spmd(nc, in_maps, list(range(NCORES)), trace=trace)
    LAST_EXEC_NS[0] = res.exec_time_ns

    out = np.empty((B, N, DIM), np.float32)
    for c in range(NCORES):
        oc = res.results[c]["out"].reshape(B, RPC, DIM)
        out[:, rows_all[c]] = oc
    return out


# revision 20
# speedup vs baseline: 1.4430x; 1.0376x over previous
"""Fused multi-query attention block (LN -> QKV -> null-token causal masked
attention -> Wo -> LN) on 8 Trainium2 NeuronCores.

Sharding: data-parallel over query rows, interleaved mod 8 so every core has
an identical causal workload (single SPMD program). Keys are compacted on the
host using the padding mask (masked keys contribute exactly zero), which
halves the attention-bias traffic — the dominant HBM stream.

Device layout: scores are computed transposed, S[j, i] (keys on partitions),
so the softmax denominator falls out of the P@V matmul via an appended
ones-column on V, and the attention output lands directly as lhsT tiles for
the Wo projection.
"""

import math
import os
import sys

sys.path.insert(0, "/opt/trn_rl_repo")

import numpy as np

import concourse.bass as bass
import concourse.tile as tile
from concourse import bacc, mybir
from concourse.masks import make_identity

B, N, DIM, H, DH = 2, 2048, 1024, 16, 64
INNER = H * DH
EPS = 1e-5
NCORES = 8
RPC = N // NCORES          # query rows per core per batch (256)
NTILES = RPC // 128        # query tiles of 128 rows per core per batch (2)
NEG = np.float16(-30000.0)

F32 = mybir.dt.float32
F16 = mybir.dt.float16

LAST_EXEC_NS = [None]


def _install_ntff_hook():
    """The image's antenv lacks axon_hooks; register it so trace=True works."""
    import types

    if "antenv.axon_hooks" in sys.modules:
        return
    try:
        import antenv
        from trn_agent_boot.trn_boot import _ntff_profile_via_ctypes
    except ImportError:
        return
    mod = types.ModuleType("antenv.axon_hooks")
    _h = [None]
    mod.set_axon_ntff_profile_hook = lambda h: _h.__setitem__(0, h)
    mod.get_axon_ntff_profile_hook = lambda: _h[0]
    sys.modules["antenv.axon_hooks"] = mod
    antenv.axon_hooks = mod
    so = "/opt/axon/libaxon_pjrt.so"
    if os.path.exists(so):
        mod.set_axon_ntff_profile_hook(_ntff_profile_via_ctypes(so))


def build_program(dim, h, kcap, ext, debug=False):
    """Build the per-core SPMD program.

    dim: model dim (mult of 128); h: heads (even); kcap: compacted key
    capacity per batch (mult of 128); ext[b][t]: key-block count per batch
    and query tile.
    """
    ncb = dim // 128                      # contraction blocks
    nkt = kcap // 128                     # key tiles per batch
    segs = [[e * 128 for e in eb] for eb in ext]   # free-len per (b, t)
    glen = [sum(s) for s in segs]                  # bias cols per (h, b) group
    sumbt = sum(glen)
    toks_q = B * RPC

    nc = bacc.Bacc()
    xq_e = nc.declare_dram_parameter("xq", [B * RPC, dim], F16, isOutput=False)
    xkv_e = nc.declare_dram_parameter("xkv", [B * kcap, dim], F16, isOutput=False)
    bias_e = nc.declare_dram_parameter("biasT", [h, 128, sumbt], F16, isOutput=False)
    wq_e = nc.declare_dram_parameter("wq", [128, ncb * h * DH], F16, isOutput=False)
    wkv_e = nc.declare_dram_parameter("wkv", [128, ncb * 2 * DH], F16, isOutput=False)
    wo_e = nc.declare_dram_parameter("wo", [128, (h // 2) * dim], F16, isOutput=False)
    nk_e = nc.declare_dram_parameter("nk", [DH, 1], F16, isOutput=False)
    nv_e = nc.declare_dram_parameter("nv1", [1, DH + 1], F16, isOutput=False)
    nb_e = nc.declare_dram_parameter("nb", [1, h], F32, isOutput=False)
    go_e = nc.declare_dram_parameter("gout", [1, dim], F32, isOutput=False)
    out_e = nc.declare_dram_parameter("out", [B * RPC, dim], F32, isOutput=True)
    dbg = {}
    if debug:
        dbg["xnt_q"] = nc.declare_dram_parameter("d_xntq", [128, ncb, B * RPC], F32, isOutput=True)
        dbg["qt0"] = nc.declare_dram_parameter("d_qt0", [128, B * RPC], F32, isOutput=True)
        dbg["kt"] = nc.declare_dram_parameter("d_kt", [128, B, kcap], F32, isOutput=True)
        dbg["v"] = nc.declare_dram_parameter("d_v", [128, B, nkt, DH + 1], F32, isOutput=True)
        dbg["aot"] = nc.declare_dram_parameter("d_aot", [128, h // 2, B * RPC], F32, isOutput=True)
        dbg["s"] = nc.declare_dram_parameter("d_s", [128, 128], F32, isOutput=True)
        dbg["p"] = nc.declare_dram_parameter("d_p", [128, 128], F32, isOutput=True)
        dbg["pso"] = nc.declare_dram_parameter("d_pso", [DH + 1, RPC], F32, isOutput=True)
        dbg["recb"] = nc.declare_dram_parameter("d_recb", [DH, RPC], F32, isOutput=True)
        dbg["bias"] = nc.declare_dram_parameter("d_bias", [128, 128], F32, isOutput=True)

    with tile.TileContext(nc) as tc:
        with (
            tc.tile_pool(name="persist", bufs=1) as pp,
            tc.tile_pool(name="work", bufs=3) as wp,
            tc.tile_pool(name="stat", bufs=4) as sp,
            tc.tile_pool(name="pwork", bufs=3, space="PSUM") as pwp,
        ):
            # ---- persistent sbuf tensors ----
            wq_sb = pp.tile([128, ncb, h * DH], F16, tag="wq")
            nc.sync.dma_start(out=wq_sb, in_=wq_e[:, :])
            wkv_sb = pp.tile([128, ncb, 2 * DH], F16, tag="wkv")
            nc.sync.dma_start(out=wkv_sb, in_=wkv_e[:, :])
            wo_sb = pp.tile([128, h // 2, dim], F16, tag="wo")
            nc.sync.dma_start(out=wo_sb, in_=wo_e[:, :])
            nk_sb = pp.tile([128, 1], F16, tag="nk")
            nc.sync.dma_start(out=nk_sb[0:DH, :], in_=nk_e[:, :])
            nc.sync.dma_start(out=nk_sb[DH : 2 * DH, :], in_=nk_e[:, :])
            nv_sb = pp.tile([1, DH + 1], F16, tag="nv")
            nc.sync.dma_start(out=nv_sb, in_=nv_e[:, :])
            nb_sb = pp.tile([1, h], F32, tag="nb")
            nc.sync.dma_start(out=nb_sb, in_=nb_e[:, :])
            go_sb = pp.tile([1, dim], F32, tag="go")
            nc.sync.dma_start(out=go_sb, in_=go_e[:, :])
            gob_sb = pp.tile([128, dim], F32, tag="gob")
            nc.gpsimd.partition_broadcast(gob_sb, go_sb)
            ident = pp.tile([128, 128], F16, tag="ident")
            make_identity(nc, ident)
            eps_sb = pp.tile([128, 1], F32, tag="eps")
            nc.vector.memset(eps_sb, EPS)

            xnt_q = pp.tile([128, ncb, toks_q], F16, tag="xntq")
            xnt_kv = pp.tile([128, ncb, B * kcap], F16, tag="xntkv")
            qt = [pp.tile([128, 2 * RPC], F16, tag=f"qt{i}", name=f"qt{i}")
                  for i in range(h // 2)]
            kt_sb = pp.tile([128, B, kcap], F16, tag="kt")
            v_sb = pp.tile([128, B, nkt, DH + 1], F16, tag="v")
            aot = pp.tile([128, h // 2, toks_q], F16, tag="aot")

            # ---- phase 1: input layernorm + transpose ----
            def ln_transpose(src_e, n_tiles, xnt_dst):
                for t in range(n_tiles):
                    x_sb = wp.tile([128, dim], F16, tag="ln_x")
                    nc.sync.dma_start(out=x_sb, in_=src_e[t * 128 : (t + 1) * 128, :])
                    gsz = math.gcd(512, dim)
                    st = sp.tile([128, dim // gsz, nc.vector.BN_STATS_DIM], F32, tag="ln_st")
                    for g in range(dim // gsz):
                        nc.vector.bn_stats(
                            out=st[:, g, :], in_=x_sb[:, g * gsz : (g + 1) * gsz]
                        )
                    mv = sp.tile([128, nc.vector.BN_AGGR_DIM], F32, tag="ln_mv")
                    nc.vector.bn_aggr(out=mv, in_=st)
                    rstd = sp.tile([128, 1], F32, tag="ln_rstd")
                    nc.scalar.activation(
                        out=rstd, in_=mv[:, 1:2],
                        func=mybir.ActivationFunctionType.Sqrt,
                        bias=eps_sb, scale=1.0,
                    )
                    nc.vector.reciprocal(out=rstd, in_=rstd)
                    nmr = sp.tile([128, 1], F32, tag="ln_nmr")
                    nc.vector.tensor_mul(nmr, mv[:, 0:1], rstd)
                    nc.vector.tensor_scalar_mul(nmr, nmr, -1.0)
                    xn = wp.tile([128, dim], F16, tag="ln_xn")
                    nc.scalar.activation(
                        out=xn, in_=x_sb,
                        func=mybir.ActivationFunctionType.Identity,
                        bias=nmr, scale=rstd,
                    )
                    for cb in range(ncb):
                        tp = pwp.tile([128, 128], F16, tag="pw")
                        nc.tensor.transpose(tp, xn[:, cb * 128 : (cb + 1) * 128], ident)
                        nc.scalar.copy(
                            xnt_dst[:, cb, t * 128 : (t + 1) * 128], tp
                        )

            ln_transpose(xq_e, toks_q // 128, xnt_q)
            ln_transpose(xkv_e, B * kcap // 128, xnt_kv)

            # ---- phase 2: projections ----
            for hp in range(h // 2):
                psq = pwp.tile([128, toks_q], F32, tag="pw")
                for cb in range(ncb):
                    nc.tensor.matmul(
                        psq,
                        wq_sb[:, cb, hp * 128 : (hp + 1) * 128],
                        xnt_q[:, cb, :],
                        start=(cb == 0), stop=(cb == ncb - 1),
                    )
                nc.vector.tensor_copy(qt[hp], psq)
            for b in range(B):
                for ch in range((kcap + 511) // 512):
                    w = min(512, kcap - ch * 512)
                    psk = pwp.tile([DH, 512], F32, tag="pw")
                    for cb in range(ncb):
                        nc.tensor.matmul(
                            psk[:, :w],
                            wkv_sb[:, cb, 0:DH],
                            xnt_kv[:, cb, b * kcap + ch * 512 : b * kcap + ch * 512 + w],
                            start=(cb == 0), stop=(cb == ncb - 1),
                        )
                    nc.vector.tensor_copy(
                        kt_sb[0:DH, b, ch * 512 : ch * 512 + w], psk[:, :w]
                    )
                for jt in range(nkt):
                    psv = pwp.tile([128, DH], F32, tag="pw")
                    for cb in range(ncb):
                        nc.tensor.matmul(
                            psv,
                            xnt_kv[:, cb, b * kcap + jt * 128 : b * kcap + (jt + 1) * 128],
                            wkv_sb[:, cb, DH : 2 * DH],
                            start=(cb == 0), stop=(cb == ncb - 1),
                        )
                    nc.vector.tensor_copy(v_sb[:, b, jt, 0:DH], psv)
            nc.vector.memset(v_sb[:, :, :, DH : DH + 1], 1.0)
            # duplicate kT into partitions 64..127 (odd heads read there)
            nc.sync.dma_start(out=kt_sb[DH : 2 * DH, :, :], in_=kt_sb[0:DH, :, :])

            # ---- phase 3: attention ----
            _pap_cm = tc.tile_pool(name="pacc", bufs=3, space="PSUM")
            pap = _pap_cm.__enter__()
            boff = [0]
            for b_ in range(B):
                boff.append(boff[-1] + glen[b_])
            for hi in range(h):
                hp, par = hi // 2, (hi % 2) * DH
                for b in range(B):
                    ext_lo, ext_hi = ext[b][0], ext[b][NTILES - 1]
                    bias_sb = wp.tile([128, max(glen)], F16, tag="bias")
                    nc.sync.dma_start(
                        out=bias_sb[:, : glen[b]],
                        in_=bias_e[hi, :, boff[b] : boff[b] + glen[b]],
                    )
                    pso = pap.tile([DH + 1, RPC], F32, tag="pso")
                    entries = []
                    for kb in range(ext_hi):
                        shared = kb < ext_lo
                        entries.append((0 if shared else 128,
                                        256 if shared else 128, kb))
                    off = 0
                    i = 0
                    while i < len(entries):
                        chunk = []
                        cu = 0
                        while i < len(entries) and cu + entries[i][1] <= 512:
                            chunk.append((entries[i], cu))
                            cu += entries[i][1]
                            i += 1
                        pss = pwp.tile([128, 512], F32, tag="pw")
                        for ci, ((qoff, w, kb), co) in enumerate(chunk):
                            nc.tensor.matmul(
                                pss[:, co : co + w],
                                kt_sb[par : par + DH, b, kb * 128 : (kb + 1) * 128],
                                qt[hp][par : par + DH,
                                       b * RPC + qoff : b * RPC + qoff + w],
                                start=(co == 0), stop=(ci == len(chunk) - 1),
                                skip_group_check=True,
                            )
                        nc.vector.tensor_add(
                            pss[:, :cu], pss[:, :cu],
                            bias_sb[:, off : off + cu],
                        )
                        p_sb = wp.tile([128, 512], F16, tag="p")
                        nc.scalar.activation(
                            out=p_sb[:, :cu], in_=pss[:, :cu],
                            func=mybir.ActivationFunctionType.Exp,
                        )
                        for (qoff, w, kb), co in chunk:
                            nc.tensor.matmul(
                                pso[:, qoff : qoff + w],
                                v_sb[:, b, kb, :],
                                p_sb[:, co : co + w],
                                start=(kb == 0), stop=False,
                                skip_group_check=True,
                            )
                        off += cu
                    # null token: S_null = q . nk + nb[h]
                    psn = pwp.tile([1, RPC], F32, tag="pw")
                    nc.tensor.matmul(
                        psn,
                        nk_sb[par : par + DH, :],
                        qt[hp][par : par + DH, b * RPC : (b + 1) * RPC],
                        start=True, stop=True,
                    )
                    pn_sb = wp.tile([1, RPC], F16, tag="pn")
                    nc.scalar.activation(
                        out=pn_sb, in_=psn,
                        func=mybir.ActivationFunctionType.Exp,
                        bias=nb_sb[0:1, hi : hi + 1],
                    )
                    nc.tensor.matmul(
                        pso, nv_sb, pn_sb,
                        start=False, stop=True,
                        skip_group_check=True,
                    )
                    # normalize by the ones-column sum and store as Wo lhsT
                    den = sp.tile([1, RPC], F32, tag="den")
                    nc.vector.tensor_copy(den, pso[DH : DH + 1, :])
                    rec = sp.tile([1, RPC], F32, tag="rec")
                    nc.vector.reciprocal_approx_fast(out=rec, in_=den)
                    recb = wp.tile([DH, RPC], F32, tag="recb")
                    nc.gpsimd.partition_broadcast(recb, rec)
                    if debug and hi == 0 and b == 1:
                        _dp = wp.tile([DH + 1, RPC], F32, tag="dbg_pso")
                        nc.vector.tensor_copy(_dp, pso)
                        nc.gpsimd.dma_start(out=dbg["pso"][:, :], in_=_dp)
                        nc.gpsimd.dma_start(out=dbg["recb"][:, :], in_=recb)
                    if par == 0:
                        nc.vector.tensor_mul(
                            aot[0:DH, hp, b * RPC : (b + 1) * RPC],
                            pso[0:DH, :], recb,
                        )
                    else:
                        tmp = wp.tile([DH, RPC], F16, tag="aotmp")
                        nc.vector.tensor_mul(tmp, pso[0:DH, :], recb)
                        nc.sync.dma_start(
                            out=aot[DH : 2 * DH, hp, b * RPC : (b + 1) * RPC], in_=tmp
                        )

            # ---- phase 4: Wo projection + output layernorm ----
            _pap_cm.__exit__(None, None, None)
            ndc = max(1, dim // 512)
            dcw = dim // ndc
            _pfp_cm = tc.tile_pool(name="pfin", bufs=2, space="PSUM")
            pfp = _pfp_cm.__enter__()
            for it in range(toks_q // 128):
                psf = [pfp.tile([128, dcw], F32, tag=f"psf{dc}", name=f"psf{dc}") for dc in range(ndc)]
                for dc in range(ndc):
                    for hp in range(h // 2):
                        nc.tensor.matmul(
                            psf[dc],
                            aot[:, hp, it * 128 : (it + 1) * 128],
                            wo_sb[:, hp, dc * dcw : (dc + 1) * dcw],
                            start=(hp == 0), stop=(hp == h // 2 - 1),
                        )
                st = sp.tile([128, ndc, nc.vector.BN_STATS_DIM], F32, tag="f_st")
                for dc in range(ndc):
                    nc.vector.bn_stats(out=st[:, dc, :], in_=psf[dc])
                mv = sp.tile([128, nc.vector.BN_AGGR_DIM], F32, tag="f_mv")
                nc.vector.bn_aggr(out=mv, in_=st)
                rstd = sp.tile([128, 1], F32, tag="f_rstd")
                nc.scalar.activation(
                    out=rstd, in_=mv[:, 1:2],
                    func=mybir.ActivationFunctionType.Sqrt,
                    bias=eps_sb, scale=1.0,
                )
                nc.vector.reciprocal(out=rstd, in_=rstd)
                for dc in range(ndc):
                    y = wp.tile([128, dcw], F32, tag="f_y")
                    nc.vector.tensor_scalar(
                        out=y, in0=psf[dc],
                        scalar1=mv[:, 0:1], scalar2=rstd,
                        op0=mybir.AluOpType.subtract, op1=mybir.AluOpType.mult,
                    )
                    nc.vector.tensor_mul(y, y, gob_sb[:, dc * dcw : (dc + 1) * dcw])
                    nc.sync.dma_start(
                        out=out_e[it * 128 : (it + 1) * 128, dc * dcw : (dc + 1) * dcw],
                        in_=y,
                    )
            _pfp_cm.__exit__(None, None, None)
            if debug:
                for name, src_t in (("xnt_q", xnt_q), ("qt0", qt[0]), ("kt", kt_sb),
                                    ("v", v_sb), ("aot", aot)):
                    sl = tuple(slice(None) for _ in src_t.shape)
                    nc.gpsimd.dma_start(out=dbg[name][sl], in_=src_t[sl])
    nc.finalize()
    return nc


def _pack_inputs(x, attn_bias, mask, g_in, null_attn_bias, null_kv, Wq, Wkv, Wo,
                 g_out, dim, h):
    """Host-side sharding/packing. Returns (in_maps, kcap, ext, unshard-info)."""
    ncb = dim // 128
    mask = np.asarray(mask)
    x = np.asarray(x, dtype=np.float32)

    keptj = [np.nonzero(mask[b])[0] + 1 for b in range(B)]  # orig key ids, 1-based
    ktot = [len(k) for k in keptj]
    kcap = max(128, int(np.ceil(max(ktot) / 128)) * 128)
    nkt = kcap // 128

    # extents per (batch, qtile): causal reach of the last row owned by any core
    ext = []
    for b in range(B):
        row = []
        for t in range(NTILES):
            imax = (t * 128 + 127) * NCORES + (NCORES - 1)
            vis = int(np.searchsorted(keptj[b], imax + 1, side="right"))
            row.append(max(1, min(nkt, int(np.ceil(vis / 128)) or 1)))
        ext.append(row)
    segs = [[e * 128 for e in eb] for eb in ext]
    glen = [sum(s) for s in segs]
    sumbt = sum(glen)

    # x_kv: compacted key tokens (key j -> token j-1), padded to kcap
    xkv = np.zeros((B, kcap, dim), np.float16)
    for b in range(B):
        xkv[b, : ktot[b]] = x[b, keptj[b] - 1].astype(np.float16)

    # masked/causal bias per batch in compacted key space, fp16
    # abm[b][hh, i, g]
    abm = []
    for b in range(B):
        ab = np.full((h, N, kcap), NEG, np.float16)
        g = attn_bias[:, :, keptj[b] - 1].astype(np.float16)  # [h, N, ktot]
        ok = keptj[b][None, :] <= (np.arange(N)[:, None] + 1)  # [N, ktot]
        ab[:, :, : ktot[b]] = np.where(ok[None], g, NEG)
        abm.append(ab)

    scale = np.float32(DH ** -0.5)
    wq_pk = (np.asarray(g_in)[:, None] * np.asarray(Wq) * scale).astype(np.float16)
    wq_pk = wq_pk.reshape(ncb, 128, h * DH).transpose(1, 0, 2).reshape(128, -1)
    wkv_pk = (np.asarray(g_in)[:, None] * np.asarray(Wkv)).astype(np.float16)
    wkv_pk = wkv_pk.reshape(ncb, 128, 2 * DH).transpose(1, 0, 2).reshape(128, -1)
    wo_pk = np.asarray(Wo).astype(np.float16)
    wo_pk = (wo_pk.reshape(h // 2, 2, DH, dim).transpose(1, 2, 0, 3)
             .reshape(128, -1))
    nv1 = np.concatenate([np.asarray(null_kv[1]), [1.0]]).astype(np.float16)[None]
    nk = np.asarray(null_kv[0]).astype(np.float16)[:, None]
    nb = np.asarray(null_attn_bias, dtype=np.float32)[None]
    gout = np.asarray(g_out, dtype=np.float32)[None]

    in_maps = []
    rows_all = []
    for c in range(NCORES):
        rows = np.arange(RPC) * NCORES + c          # per batch, ascending
        rows_all.append(rows)
        xq = x[:, rows].astype(np.float16).reshape(B * RPC, dim)
        biasT = np.empty((h, 128, sumbt), np.float16)
        for hh in range(h):
            o = 0
            for b in range(B):
                ext_lo, ext_hi = ext[b][0], ext[b][NTILES - 1]
                for kb in range(ext_hi):
                    taus = ([0, 1] if kb < ext_lo else [1])
                    for t in taus:
                        blkc = abm[b][hh][rows[t * 128 : (t + 1) * 128],
                                          kb * 128 : (kb + 1) * 128]
                        biasT[hh, :, o : o + 128] = blkc.T
                        o += 128
            assert o == sumbt
        in_maps.append({
            "xq": xq, "xkv": xkv.reshape(B * kcap, dim), "biasT": biasT,
            "wq": wq_pk, "wkv": wkv_pk, "wo": wo_pk,
            "nk": nk, "nv1": nv1, "nb": nb, "gout": gout,
        })
    return in_maps, kcap, ext, rows_all


def kernel(x, attn_bias, mask, g_in, null_attn_bias, null_kv, Wq, Wkv, Wo, g_out):
    _install_ntff_hook()
    from concourse.bass_utils import run_bass_kernel_spmd

    in_maps, kcap, ext, rows_all = _pack_inputs(
        x, attn_bias, mask, g_in, null_attn_bias, null_kv, Wq, Wkv, Wo, g_out,
        DIM, H,
    )
    nc = build_program(DIM, H, kcap, ext)
    trace = os.environ.get("KERNEL_TRACE", "0") == "1"
    res = run_bass_kernel_spmd(nc, in_maps, list(range(NCORES)), trace=trace)
    LAST_EXEC_NS[0] = res.exec_time_ns

    out = np.empty((B, N, DIM), np.float32)
    for c in range(NCORES):
        oc = res.results[c]["out"].reshape(B, RPC, DIM)
        out[:, rows_all[c]] = oc
    return out
